# revision 71
# baseline (speedup 1.0000x reference)
"""Cross multihead attention (global/local masked head groups) on 8 trn2 cores.

Sharding: core c -> (batch b = c//2, head-group g = c%2).
  g=0: heads 0-7  masked by key_padding_mask[b]
  g=1: heads 8-15 masked by local_mask[b]
Each core computes its group's partial output (attn_out_g @ Wo[:, gs].T) of
shape [T, E]; the host sums the two partials per batch and adds bo + bv@Wo.T
(the v-bias contributes a constant row because softmax rows sum to 1).

Key optimizations over the dense baseline:
  * The mask is a per-core constant, so unmasked key/value positions are
    gathered on the host: S shrinks from 1024 to ~549, padded to 640 (5
    s-tiles). Halves k/v projection, scores, exp, and AV work.
  * AV is computed in [t, d] orientation (lhsT = exp scores tile, rhs = v
    stripes with a ones column). The softmax denominator lands in column 64
    as a per-partition scalar, so normalization is one tensor_scalar divide -
    no cross-partition broadcast matmuls.
  * Normalized head-pair tiles reach feature-major aT via DMA-engine xbar
    transposes (dma_start_transpose) into standalone contiguous [128,128]
    tiles - no PE transposes, no PSUM->SBUF copies, no identity matrix.
  * Input DMAs are ordered, split by first use, and spread across the SP /
    ACT / Pool DMA queues so transfers overlap: the first q-proj matmul
    starts ~2.4us in (was ~8) and k-proj never starves on xk.
  * Per-t4 reciprocals + normalizes are fused into the AV matmul stream, so
    nothing downstream waits on a recip->norm chain and the AV PSUM pool
    never serializes consecutive head pairs (the psum's last reader is the
    norm, which lands mid-stream).
  * Scores (PE, ~213ns/tile) and exp (ACT, ~630ns/tile) are zip-scheduled
    against projection/AV/out matmuls so neither engine stalls the other;
    AV c0 head pairs start as soon as the v stripes exist (spreading DVE
    norms through phase B), AV c1 splits its norms across ACT and DVE, and
    out c0 groups pace the AV c1 chains. The Exp table is preloaded during
    the DMA lead-in.
  * Output is written as bf16 (half the out-DMA bytes); the last row-tile
    drains in 256-col psum groups across engines/queues to shorten the
    final copy->DMA->sem chain. Host sums partials in fp32.

  * k-proj covers only the real gathered keys (se_pad ~549 of 640); the
    padded kT tail is memset so exp never sees uninitialized SBUF.

CoreSim cost-model estimate: 75952 ns/core (baseline kernel: 92509).
"""

import sys

sys.path.insert(0, "/opt/trn_rl_repo")

import numpy as np

import concourse.bass as bass
import concourse.mybir as mybir
from concourse.tile import TileContext

B, T, S, E, H = 4, 1024, 1024, 1024, 16
DH = E // H            # 64
HH = H // 2            # 8 heads per group
G = HH * DH            # 512 features per group
SCALING = DH ** -0.5
NEG = -30000.0         # exp(x + NEG) == 0.0 in fp32, no LUT edge cases
ET = E // 128          # 8 e-tiles
JT = G // 128          # 4 j-tiles
TC = 2                 # t-chunks of 512

F32 = mybir.dt.float32
BF = mybir.dt.bfloat16   # tensor-engine operand dtype (1 cyc/row)


def _split_waits(nc):
    """TPB ISA structs hold one sem-wait slot. Tile can emit >1 wait per
    instruction (walrus: 'Too many sync wait commands'); hoist all but the
    last wait onto single-wait NOPs on the same engine, inserted just
    before. Timing is unchanged - the waits would have blocked anyway."""
    k = 0
    for f in nc.m.functions:
        for blk in f.blocks:
            new = []
            for inst in blk.instructions:
                si = inst.sync_info
                w = list(si.on_wait) if si else []
                if len(w) > 1:
                    for wait in w[:-1]:
                        nop = mybir.InstNoOp(name=f"nopw-{k}", ins=[], outs=[])
                        k += 1
                        nop.engine = inst.engine
                        nop.sync_info = mybir.SyncInfo(on_wait=[wait], on_update=[])
                        new.append(nop)
                    inst.sync_info = mybir.SyncInfo(
                        on_wait=[w[-1]], on_update=list(si.on_update)
                    )
                new.append(inst)
            blk.instructions = new
    return nc


def build_nc(st_tiles=5, se_pad=None, split=True):
    ST = st_tiles
    SP = 128 * ST
    if se_pad is None:
        se_pad = SP
    nc = bass.Bass()

    xq0 = nc.dram_tensor("xq0", [128, ET * 512], BF, kind="ExternalInput")
    xq1 = nc.dram_tensor("xq1", [128, ET * 512], BF, kind="ExternalInput")
    xk = nc.dram_tensor("xk", [128, ET * SP], BF, kind="ExternalInput")
    xv = nc.dram_tensor("xv", [128, ET * SP], BF, kind="ExternalInput")
    wq = nc.dram_tensor("wq", [128, ET * G], BF, kind="ExternalInput")
    wk = nc.dram_tensor("wk", [128, ET * G], BF, kind="ExternalInput")
    wv = nc.dram_tensor("wv", [128, ET * G], BF, kind="ExternalInput")
    wo = nc.dram_tensor("wo", [128, JT * E], BF, kind="ExternalInput")
    mb = nc.dram_tensor("mb", [128, ST], F32, kind="ExternalInput")
    bqc = nc.dram_tensor("bqc", [128, JT], F32, kind="ExternalInput")
    bkc = nc.dram_tensor("bkc", [128, JT], F32, kind="ExternalInput")
    out = nc.dram_tensor("out", [T, E], BF, kind="ExternalOutput")

    # k-proj only covers real keys [0, se_pad); kT's padded tail is
    # memset to zero (mb's NEG bias would kill those exps anyway, but
    # uninitialized SBUF could overflow the exp input)
    # a thin first chunk gets kT's first st-tile drained after ~0.5us of
    # k matmuls instead of ~3.4, so the exp stream starts ~3us earlier
    schunks = [(0, 128)] + [(o, min(512, se_pad - o))
                            for o in range(128, se_pad, 512)]
    W = DH + 1

    with TileContext(nc) as tc:
        with (
            tc.tile_pool(name="const", bufs=1) as pc,
            tc.tile_pool(name="persist", bufs=1) as pp,
            tc.tile_pool(name="exp", bufs=2 * HH * ST) as pe,
            tc.tile_pool(name="pair", bufs=12) as ppr,
            tc.tile_pool(name="outsb", bufs=3) as po,
            tc.tile_pool(name="small", bufs=8) as psm,
            tc.tile_pool(name="psg", bufs=3, space="PSUM") as ppsg,
            tc.tile_pool(name="pssc", bufs=2, space="PSUM") as ppsc,
            tc.tile_pool(name="psav", bufs=3, space="PSUM") as ppsav,
        ):
            # ---- bulk inputs, ordered and split by first use. Small consts
            # ride the Pool queue so they never hold up the SP stream. ----
            RW = ET * 128   # columns per r-quarter of an r-major weight image
            HQ = ET * 256   # half of an xq image
            HK = ET * SP // 2
            wq_sb = pp.tile([128, ET * G], BF, name="wq_sb")
            nc.sync.dma_start(out=wq_sb[:, 0:256], in_=wq[:, 0:256])
            xq0_sb = pp.tile([128, ET * 512], BF, name="xq0_sb")
            # first xq0 piece on the ACT queue, concurrent with wq on SP
            nc.scalar.dma_start(out=xq0_sb[:, 0:512], in_=xq0[:, 0:512])
            nc.sync.dma_start(out=wq_sb[:, 256:RW], in_=wq[:, 256:RW])
            nc.sync.dma_start(out=xq0_sb[:, 512:HQ // 2], in_=xq0[:, 512:HQ // 2])
            nc.sync.dma_start(out=xq0_sb[:, HQ // 2:HQ], in_=xq0[:, HQ // 2:HQ])
            nc.scalar.dma_start(out=xq0_sb[:, HQ:], in_=xq0[:, HQ:])
            wk_sb = pp.tile([128, ET * G], BF, name="wk_sb")
            nc.sync.dma_start(out=wk_sb[:, 0:RW], in_=wk[:, 0:RW])
            # xk/xv ride the otherwise-idle Pool DMA queue, overlapping the
            # SP queue's q-path loads - k-proj never starves on xk
            xk_sb = pp.tile([128, ET * SP], BF, name="xk_sb")
            nc.gpsimd.dma_start(out=xk_sb[:, 0:HK], in_=xk[:, 0:HK])
            nc.gpsimd.dma_start(out=xk_sb[:, HK:], in_=xk[:, HK:])
            # biases/mask ride the ACT queue behind xq0b: they land by ~4us,
            # before the first q/k drains and the first exp need them
            bq_sb = pc.tile([128, JT], F32, name="bq_sb")
            nc.scalar.dma_start(out=bq_sb[:], in_=bqc[:])
            mb_sb = pc.tile([128, ST], F32, name="mb_sb")
            nc.scalar.dma_start(out=mb_sb[:], in_=mb[:])
            bk_sb = pc.tile([128, JT], F32, name="bk_sb")
            nc.scalar.dma_start(out=bk_sb[:], in_=bkc[:])
            xv_sb = pp.tile([128, ET * SP], BF, name="xv_sb")
            nc.gpsimd.dma_start(out=xv_sb[:, 0:HK], in_=xv[:, 0:HK])
            nc.gpsimd.dma_start(out=xv_sb[:, HK:], in_=xv[:, HK:])
            # preload the Exp activation table during the DMA lead-in so the
            # first real exp doesn't pay the ~1.3us table load
            warm = psm.tile([128, 1], F32, tag="warm", name="warm")
            nc.scalar.activation(warm[:], mb_sb[:, 0:1],
                                 mybir.ActivationFunctionType.Exp, scale=0.0)
            # per-r quarters so q(r,0)/k(r) never wait on a monolithic load
            for r in (1, 2, 3):
                nc.sync.dma_start(out=wq_sb[:, r * RW:(r + 1) * RW],
                                  in_=wq[:, r * RW:(r + 1) * RW])
                nc.sync.dma_start(out=wk_sb[:, r * RW:(r + 1) * RW],
                                  in_=wk[:, r * RW:(r + 1) * RW])
            xq1_sb = pp.tile([128, ET * 512], BF, name="xq1_sb")
            nc.sync.dma_start(out=xq1_sb[:], in_=xq1[:])
            wv_sb = pp.tile([128, ET * G], BF, name="wv_sb")
            nc.sync.dma_start(out=wv_sb[:], in_=wv[:])
            wo_sb = pp.tile([128, JT * E], BF, name="wo_sb")
            nc.sync.dma_start(out=wo_sb[:], in_=wo[:])

            # ---- persistent activations ----
            qT = [pp.tile([128, T], BF, name=f"qT{r}") for r in range(JT)]
            kT = [pp.tile([128, SP], BF, name=f"kT{r}") for r in range(JT)]
            if se_pad < SP:
                for r in range(JT):
                    nc.gpsimd.memset(kT[r][:, se_pad:SP], 0.0)
            v_sb = [pp.tile([128, HH * W], BF, name=f"v{st}") for st in range(ST)]
            # one standalone [128,128] tile per (feature-row-tile, t-tile):
            # dma_start_transpose needs a CONTIGUOUS destination (a strided
            # slice of a wider tile gives wrong output on hardware)
            aT = {(r, tt): pp.tile([128, 128], BF, name=f"aT{r}_{tt}")
                  for r in range(JT) for tt in range(2 * 4)}

            exp_sb = {}  # (c, h, st) -> tile, allocated on demand

            # ---------- emission-item constructors ----------
            def k_proj_items(r):
                items = []
                state = {}
                for co, csz in schunks:
                    for et in range(ET):
                        def mm(r=r, co=co, csz=csz, et=et):
                            if et == 0:
                                state[co] = ppsg.tile([128, csz], F32, tag="psg",
                                                      name=f"psk{r}_{co}")
                            ps = state[co]
                            nc.tensor.matmul(
                                ps[:],
                                lhsT=wk_sb[:, r * RW + et * 128:r * RW + (et + 1) * 128],
                                rhs=xk_sb[:, et * SP + co:et * SP + co + csz],
                                start=(et == 0), stop=(et == ET - 1),
                            )
                            if et == ET - 1:
                                # split drain: scores st-tiles unblock sooner
                                for o in range(0, csz, 256):
                                    w = min(256, csz - o)
                                    nc.vector.tensor_scalar_add(
                                        kT[r][:, co + o:co + o + w],
                                        ps[:, o:o + w], bk_sb[:, r:r + 1],
                                    )
                        items.append(mm)
                return items

            def q_proj_items(r, c):
                items = []
                xsrc = xq0_sb if c == 0 else xq1_sb
                state = {}
                for et in range(ET):
                    def mm(r=r, c=c, et=et):
                        if et == 0:
                            state["ps"] = ppsg.tile([128, 512], F32, tag="psg",
                                                    name=f"psq{r}_{c}")
                        ps = state["ps"]
                        nc.tensor.matmul(
                            ps[:],
                            lhsT=wq_sb[:, r * RW + et * 128:r * RW + (et + 1) * 128],
                            rhs=xsrc[:, et * 512:(et + 1) * 512],
                            start=(et == 0), stop=(et == ET - 1),
                        )
                        if et == ET - 1:
                            nc.vector.tensor_scalar_add(
                                qT[r][:, c * 512:(c + 1) * 512], ps[:],
                                bq_sb[:, r:r + 1],
                            )
                    items.append(mm)
                return items

            def v_proj_items(st):
                items = []
                state = {}
                for et in range(ET):
                    def mm(st=st, et=et):
                        if et == 0:
                            state["ps"] = ppsg.tile([128, G], F32, tag="psg",
                                                    name=f"psv{st}")
                        ps = state["ps"]
                        nc.tensor.matmul(
                            ps[:],
                            lhsT=xv_sb[:, et * SP + st * 128:et * SP + (st + 1) * 128],
                            rhs=wv_sb[:, et * G:(et + 1) * G],
                            start=(et == 0), stop=(et == ET - 1),
                        )
                        if et == ET - 1:
                            # GPSIMD cannot read PSUM on HW - scatter on DVE
                            v3 = v_sb[st][:].rearrange("p (h x) -> p h x", x=W)
                            nc.vector.tensor_copy(
                                v3[:, :, 0:DH],
                                ps[:].rearrange("p (h x) -> p h x", x=DH),
                            )
                            nc.gpsimd.memset(v3[:, :, DH:W], 1.0)
                    items.append(mm)
                return items

            def score_items(r, c):
                """scores + exp for heads (2r, 2r+1), chunk c: 2*ST items."""
                items = []
                for st in range(ST):
                    for h in (2 * r, 2 * r + 1):
                        def mm(r=r, c=c, st=st, h=h):
                            po_ = (h % 2) * DH
                            ps_s = ppsc.tile([128, 512], F32, tag="sc", name="ps_s")
                            nc.tensor.matmul(
                                ps_s[:],
                                lhsT=kT[r][po_:po_ + DH, st * 128:(st + 1) * 128],
                                rhs=qT[r][po_:po_ + DH, c * 512:(c + 1) * 512],
                                start=True, stop=True,
                            )
                            ex = pe.tile([128, 512], BF, tag="exp",
                                         name=f"exp{c}_{h}_{st}")
                            exp_sb[(c, h, st)] = ex
                            nc.scalar.activation(
                                ex[:], ps_s[:],
                                mybir.ActivationFunctionType.Exp,
                                bias=mb_sb[:, st:st + 1], scale=SCALING,
                            )
                        items.append(mm)
                return items

            def av_items(hp, c):
                """AV matmuls for head pair hp, chunk c. The (h, t4) chain's
                stop-matmul also emits that t4's reciprocal and normalize
                (DVE), so the psum's last reader lands mid-stream: transposes
                never wait on a recip chain and the 2-buf psav pool never
                serializes consecutive head pairs."""
                items = []
                state = {}
                for h in (2 * hp, 2 * hp + 1):
                    for t4 in range(4):
                        for st in range(ST):
                            def mm(c=c, h=h, t4=t4, st=st):
                                if t4 == 0 and st == 0:
                                    state[h] = ppsav.tile([128, 4 * W], F32,
                                                          tag="av", name=f"psav{h}")
                                    state[("rc", h)] = psm.tile(
                                        [128, 4], F32, tag="rc", name=f"rc{c}_{h}")
                                ps4 = state[h]
                                nc.tensor.matmul(
                                    ps4[:, t4 * W:(t4 + 1) * W],
                                    lhsT=exp_sb[(c, h, st)][:, t4 * 128:(t4 + 1) * 128],
                                    rhs=v_sb[st][:, h * W:(h + 1) * W],
                                    start=(st == 0), stop=(st == ST - 1),
                                )
                                if st == ST - 1:
                                    rc = state[("rc", h)]
                                    nc.vector.reciprocal(
                                        rc[:, t4:t4 + 1],
                                        ps4[:, t4 * W + DH:t4 * W + DH + 1],
                                    )
                                    if h % 2 == 0:
                                        state[("pair", t4)] = ppr.tile(
                                            [128, 128], BF, tag="pair",
                                            name=f"pr{4 * c + t4}")
                                    pair = state[("pair", t4)]
                                    if c == 1 and h % 2 == 0 and h < 6:
                                        nc.scalar.activation(
                                            pair[:, 0:DH],
                                            ps4[:, t4 * W:t4 * W + DH],
                                            mybir.ActivationFunctionType.Copy,
                                            scale=rc[:, t4:t4 + 1])
                                    else:
                                        nc.vector.tensor_scalar_mul(
                                            pair[:, (h % 2) * DH:(h % 2 + 1) * DH],
                                            ps4[:, t4 * W:t4 * W + DH],
                                            rc[:, t4:t4 + 1])
                            items.append(mm)
                return items, state

            def av_tr_items(hp, c, state):
                """Each normalized pair tile goes to aT via a DMA-engine
                transpose (xbar): no PE transpose, no PSUM->SBUF copy, and
                the latency hides behind the zipped matmul stream. Alternate
                HWDGE queues so consecutive transposes overlap."""
                items = []
                for t4 in range(4):
                    def tr(hp=hp, c=c, t4=t4):
                        tt = 4 * c + t4
                        pair = state[("pair", t4)]
                        nc.sync.dma_start_transpose(aT[(hp, tt)][:], pair[:])
                    items.append(tr)
                return items

            def out_items(tt, fine=False):
                """Both 512-wide halves of out rows tt*128..+128; one DMA per
                half as soon as it drains. fine=True (last group) accumulates
                in 256-col psum groups and drains each on its own engine and
                DMA queue, so the final copy->DMA->sem chain is as short as
                possible."""
                if fine:
                    items = []
                    state = {}
                    qs = [nc.sync, nc.gpsimd, nc.scalar, nc.sync]
                    for sub in range(4):
                        for r in range(JT):
                            def mm(tt=tt, sub=sub, r=r):
                                if sub == 0 and r == 0:
                                    state["ot"] = po.tile([128, E], BF, tag="ot",
                                                          name=f"ot{tt}")
                                if r == 0:
                                    state["ps"] = ppsg.tile(
                                        [128, 256], F32, tag="psg",
                                        name=f"psu{tt}_{sub}")
                                ps_u = state["ps"]
                                co = sub * 256
                                nc.tensor.matmul(
                                    ps_u[:],
                                    lhsT=aT[(r, tt)][:],
                                    rhs=wo_sb[:, r * E + co:r * E + co + 256],
                                    start=(r == 0), stop=(r == JT - 1),
                                )
                                if r == JT - 1:
                                    dst = state["ot"][:, co:co + 256]
                                    if sub % 2:
                                        nc.vector.tensor_copy(dst, ps_u[:])
                                    else:
                                        nc.scalar.copy(dst, ps_u[:])
                                    qs[sub].dma_start(
                                        out=out[tt * 128:(tt + 1) * 128,
                                                co:co + 256],
                                        in_=dst,
                                    )
                            items.append(mm)
                    return items
                items = []
                state = {}
                for oc in range(2):
                    for r in range(JT):
                        def mm(tt=tt, oc=oc, r=r):
                            if oc == 0 and r == 0:
                                state["ot"] = po.tile([128, E], BF, tag="ot",
                                                      name=f"ot{tt}")
                            if r == 0:
                                state["ps"] = ppsg.tile([128, 512], F32, tag="psg",
                                                        name=f"psu{tt}_{oc}")
                            ps_u = state["ps"]
                            nc.tensor.matmul(
                                ps_u[:],
                                lhsT=aT[(r, tt)][:],
                                rhs=wo_sb[:, r * E + oc * 512:r * E + (oc + 1) * 512],
                                start=(r == 0), stop=(r == JT - 1),
                            )
                            if r == JT - 1:
                                # split tail copies across ACT (exp stream
                                # done) and DVE so neither queue backs up
                                ot = state["ot"]
                                dst = ot[:, oc * 512:(oc + 1) * 512]
                                if tt >= 4 and oc == 0:
                                    nc.scalar.copy(dst, ps_u[:])
                                else:
                                    nc.vector.tensor_copy(dst, ps_u[:])
                                # late groups spread across DMA queues so the
                                # tail never queues behind SP
                                dq = nc.gpsimd if (tt >= 6 and oc == 1) else nc.sync
                                dq.dma_start(
                                    out=out[tt * 128:(tt + 1) * 128,
                                            oc * 512:(oc + 1) * 512],
                                    in_=dst,
                                )
                        items.append(mm)
                return items

            # ---------- schedule ----------
            def zip_emit(filler, scores, ratio=2):
                """Emit filler items, inserting one scores item after every
                `ratio` filler items while scores remain."""
                n = 0
                for it in filler:
                    it()
                    n += 1
                    if scores and n % ratio == 0:
                        scores.pop(0)()
                while scores:
                    scores.pop(0)()

            s1 = []  # pending score/exp items (prereqs met when appended)

            # AV head-pair group: matmuls zipped against pending scores;
            # transposes are DMA issues, emitted right after the norms
            def av_group(hp, c):
                mms, st_av = av_items(hp, c)
                zip_emit(mms, s1, ratio=20)
                for tr in av_tr_items(hp, c, st_av):
                    tr()

            # phase A: q r0 c0 (first weight quarter + xq0 halves land
            # first), then k r0 for the first scores
            for it in q_proj_items(0, 0) + k_proj_items(0):
                it()
            s1 += score_items(0, 0)

            # phase B: remaining k/v/q-c1 zipped against scores; v-proj
            # interleaves with q c1 blocks so the exp stream never starves.
            for r in (1, 2, 3):
                zip_emit(q_proj_items(r, 0) + k_proj_items(r), s1)
                s1 += score_items(r, 0)
            zip_emit(q_proj_items(0, 1), s1)
            s1 += score_items(0, 1)
            zip_emit(v_proj_items(0) + v_proj_items(1), s1)
            zip_emit(q_proj_items(1, 1), s1)
            s1 += score_items(1, 1)
            zip_emit(v_proj_items(2) + v_proj_items(3), s1)
            zip_emit(q_proj_items(2, 1), s1)
            s1 += score_items(2, 1)
            zip_emit(v_proj_items(4) if ST > 4 else [], s1)
            for st in range(5, ST):
                zip_emit(v_proj_items(st), s1)
            # first three AV c0 head pairs run as soon as all v stripes
            # exist - their exps (c0) are long done, and moving them here
            # spreads the DVE norm load out of the phase-C crunch
            av_early = [0, 1, 2]
            av_group(0, 0)
            av_group(1, 0)
            zip_emit(q_proj_items(3, 1), s1)
            s1 += score_items(3, 1)
            av_group(2, 0)

            for hp in range(HH // 2):
                if hp in av_early:
                    continue
                av_group(hp, 0)
                if s1:
                    s1.pop(0)()
            # phase D: out c0 groups (unlocked by av c0 hp3) interleave the
            # AV c1 head pairs; a reserved out c0 group pads the last
            # transpose-DMA latency before out c1 starts.
            fill = []
            for tt in range(4):
                fill += out_items(tt)
            for hp in range(HH // 2):
                av_group(hp, 1)
                for _ in range(6):
                    if fill:
                        fill.pop(0)()
            while s1:
                s1.pop(0)()
            half = fill[:len(fill) // 2]
            rest = fill[len(fill) // 2:]
            for it in half:
                it()
            for tt in range(4, 8):
                zip_emit(out_items(tt, fine=(tt == 7)), rest, ratio=4)
            for it in rest:
                it()

    return _split_waits(nc) if split else nc


_NC_CACHE = {}


def _get_nc(st, se_pad):
    key = (st, se_pad)
    if key not in _NC_CACHE:
        _NC_CACHE[key] = build_nc(st, se_pad)
    return _NC_CACHE[key]


def _img(mat):
    """[E_or_G rows, C cols] -> SBUF image [128, (rows/128)*C], row-tile major."""
    import ml_dtypes
    r, c = mat.shape
    return np.ascontiguousarray(
        mat.reshape(r // 128, 128, c).transpose(1, 0, 2).reshape(128, -1)
    ).astype(ml_dtypes.bfloat16)


def _img_r(mat):
    """[E, G] weight -> r-major SBUF image [128, JT*ET*128]: column
    r*ET*128 + et*128 + jj holds mat[et*128 + p, r*128 + jj]."""
    import ml_dtypes
    e, g = mat.shape
    return np.ascontiguousarray(
        mat.reshape(e // 128, 128, g // 128, 128)
        .transpose(1, 2, 0, 3).reshape(128, -1)
    ).astype(ml_dtypes.bfloat16)


def make_in_maps(query, key, value, key_padding_mask, local_mask,
                 Wq, bq, Wk, bk, Wv, bv, Wo, bo):
    f = np.float32
    masks = [np.asarray(key_padding_mask), np.asarray(local_mask)]
    idxs = []
    for c in range(8):
        b, g = c // 2, c % 2
        idxs.append(np.nonzero(~masks[g][b])[0])
    st = max(5, max((len(ix) + 127) // 128 for ix in idxs))
    sp = st * 128
    kernel._se_pad = min(sp, max(len(ix) for ix in idxs))

    in_maps = []
    for c in range(8):
        b, g = c // 2, c % 2
        gs = slice(g * G, (g + 1) * G)
        ix = idxs[c]
        se = len(ix)
        kg = np.zeros((sp, E), f)
        kg[:se] = np.asarray(key[b], f)[ix]
        vg = np.zeros((sp, E), f)
        vg[:se] = np.asarray(value[b], f)[ix]
        mbias = np.full((sp,), NEG, f)
        mbias[:se] = 0.0
        qT = np.asarray(query[b], f).T
        in_maps.append({
            "xq0": _img(qT[:, 0:512]),
            "xq1": _img(qT[:, 512:1024]),
            "xk": _img(kg.T),
            "xv": _img(vg.T),
            "wq": _img_r(np.asarray(Wq, f)[gs, :].T),
            "wk": _img_r(np.asarray(Wk, f)[gs, :].T),
            "wv": _img(np.asarray(Wv, f)[gs, :].T),
            "wo": _img(np.asarray(Wo, f)[:, gs].T),
            "mb": np.ascontiguousarray(mbias.reshape(st, 128).T),
            "bqc": np.ascontiguousarray(np.asarray(bq, f)[gs].reshape(JT, 128).T),
            "bkc": np.ascontiguousarray(np.asarray(bk, f)[gs].reshape(JT, 128).T),
        })
    return in_maps


def kernel(query, key, value, key_padding_mask, local_mask,
           Wq, bq, Wk, bk, Wv, bv, Wo, bo, _trace=False, _tmpdir=None):
    from concourse.bass_utils import run_bass_kernel_spmd

    in_maps = make_in_maps(query, key, value, key_padding_mask, local_mask,
                           Wq, bq, Wk, bk, Wv, bv, Wo, bo)
    st = in_maps[0]["mb"].shape[1]
    nc = _get_nc(st, kernel._se_pad)
    try:
        res = run_bass_kernel_spmd(nc, in_maps, list(range(8)),
                                   trace=_trace, tmpdir=_tmpdir)
    except Exception:
        # transient device/transport failures have been observed on the
        # axon path; one fresh attempt is cheap relative to a hard fail
        res = run_bass_kernel_spmd(nc, in_maps, list(range(8)),
                                   trace=_trace, tmpdir=_tmpdir)
    outs = [np.asarray(r["out"]).astype(np.float32) for r in res.results]
    full = np.stack([outs[2 * b] + outs[2 * b + 1] for b in range(B)])
    full += np.asarray(bo, dtype=np.float32)
    full += np.asarray(bv, np.float32) @ np.asarray(Wo, np.float32).T
    if _trace:
        kernel._last_exec_time_ns = res.exec_time_ns
        kernel._last_profile = res.profile_json
    return full.astype(np.float32)
